# revision 1
# baseline (speedup 1.0000x reference)
"""Trainium2 Bass kernel for nn_DeformRouting (deformable routing conv).

Strategy (8 cores, data-parallel over N x H-halves):
  core c handles image n = c//2, row-half = c%2 (14 rows x 28 cols = 392 pixels).

Per-core device pipeline (points-on-partitions layout, 4 chunks of 98 pts):
  1. offset conv: 4 PE matmuls  out[pt,18] = x_chunk.T @ w_off.T
  2. coordinate math on [128, 36] tiles (DVE/ACT elementwise):
     grid coords -> floor, fractional weights, validity, clamped indices.
  3. bilinear gather: 2 indirect DMAs from a host-built 29x28 row-PAIR table
     (each gathered 512B row = [x[y0c], x[y0c+1]] stacked over 64 channels).
  4. combine: samp = w00*A0 + w10*A1 + w01*B0 + w11*B1  (DVE, free-dim
     broadcast weights);  q = samp * x  (the grouped weight-gen fold).
  5. PE transpose of the stacked [q; samp] tensor to (u,c)-on-partitions.
  6. 9 accumulating PE matmuls: out[o, pt] += Wstack_kk.T @ rhs_kk
     where Wstack_kk = [w_wgt_kk ; b_wgt_kk]  (the per-pixel matvec,
     algebraically refactored: out = sum_ck W2*x*samp + B2*samp).
"""

import numpy as np

import concourse.bass as bass
import concourse.tile as tile
from concourse import bacc, mybir
from concourse.bass import IndirectOffsetOnAxis
from concourse.bass_utils import run_bass_kernel_spmd
from concourse.masks import make_identity

# problem constants (hardcoded per contract)
N, CIN, COUT, H, W, K = 4, 64, 64, 28, 28, 3
K2 = K * K  # 9
NCORES = 8
HHALF = H // 2          # 14 rows per core
NPT = HHALF * W         # 392 points per core
PCH = 98                # points per partition-chunk
NCH = 4                 # chunks (4*98 = 392)
TBL_ROWS = (H + 1) * W  # 812 pair-table rows
SC = (W - 1) / 2.0      # 13.5

F32 = mybir.dt.float32
I32 = mybir.dt.int32

_CACHE = {}


def _alu(name):
    return getattr(mybir.AluOpType, name)


def _build_program():
    """Build + compile the (SPMD-identical) Bass program once."""
    nc = bacc.Bacc("TRN2", target_bir_lowering=False, debug=False,
                   num_devices=NCORES)

    # DRAM I/O (per-core shapes)
    xpair = nc.dram_tensor("xpair", [TBL_ROWS, 2 * CIN], F32, kind="ExternalInput")
    xcpad = nc.dram_tensor("xcpad", [128, NPT], F32, kind="ExternalInput")
    wofft = nc.dram_tensor("wofft", [128, 2 * K2], F32, kind="ExternalInput")
    basex = nc.dram_tensor("basex", [128, NCH * K2], F32, kind="ExternalInput")
    basey = nc.dram_tensor("basey", [128, NCH * K2], F32, kind="ExternalInput")
    wwb = nc.dram_tensor("wwb", [128, 10 * COUT], F32, kind="ExternalInput")
    mg = nc.dram_tensor("mg", [128, 8 * 128], F32, kind="ExternalInput")
    out_d = nc.dram_tensor("out", [COUT, NPT], F32, kind="ExternalOutput")

    mult, add, sub = _alu("mult"), _alu("add"), _alu("subtract")
    is_gt, is_eq = _alu("is_gt"), _alu("is_equal")
    amin, amax = _alu("min"), _alu("max")

    with tile.TileContext(nc) as tc:
        with (
            tc.tile_pool(name="const", bufs=1) as cpool,
            tc.tile_pool(name="work", bufs=1) as wpool,
            tc.tile_pool(name="psoff", bufs=1, space="PSUM") as opool,
            tc.tile_pool(name="psum", bufs=2, space="PSUM") as ppool,
            tc.tile_pool(name="pso", bufs=1, space="PSUM") as popool,
        ):
            # ---- load constants/inputs ----
            xc_sb = cpool.tile([128, NPT], F32)
            nc.sync.dma_start(xc_sb[:], xcpad.ap())
            wofft_sb = cpool.tile([128, 2 * K2], F32)
            nc.sync.dma_start(wofft_sb[:], wofft.ap())
            basex_sb = cpool.tile([128, NCH, K2], F32)
            nc.sync.dma_start(basex_sb[:], basex.ap().rearrange(
                "p (a b) -> p a b", a=NCH))
            basey_sb = cpool.tile([128, NCH, K2], F32)
            nc.sync.dma_start(basey_sb[:], basey.ap().rearrange(
                "p (a b) -> p a b", a=NCH))
            wwb_sb = cpool.tile([128, 10, COUT], F32)
            nc.sync.dma_start(wwb_sb[:], wwb.ap().rearrange(
                "p (a b) -> p a b", a=10))
            ident = cpool.tile([128, 128], F32)
            make_identity(nc, ident[:])
            mg_sb = cpool.tile([128, 8, 128], F32)
            nc.sync.dma_start(mg_sb[:], mg.ap().rearrange(
                "p (a b) -> p a b", a=8))

            # ---- 1. offset conv: psum[pt(98), ch, 18] ----
            ps_off = opool.tile([128, NCH, 2 * K2], F32)
            for ch in range(NCH):
                nc.tensor.matmul(
                    out=ps_off[:PCH, ch, :],
                    lhsT=xc_sb[:, ch * PCH:(ch + 1) * PCH],
                    rhs=wofft_sb[:],
                    start=True, stop=True,
                )
            offx = wpool.tile([128, NCH, K2], F32)
            offy = wpool.tile([128, NCH, K2], F32)
            nc.any.memset(offx[:], 0.0)
            nc.any.memset(offy[:], 0.0)
            for ch in range(NCH):
                nc.any.tensor_copy(offx[:PCH, ch, :], ps_off[:PCH, ch, 0:18:2])
                nc.any.tensor_copy(offy[:PCH, ch, :], ps_off[:PCH, ch, 1:18:2])

            # ---- 2. coordinate math on [128, 36] ----
            shp = [128, NCH, K2]
            _cnt = [0]

            def t(name=None):
                _cnt[0] += 1
                return wpool.tile(shp, F32, name=f"ct{_cnt[0]}")

            def floor_of(i_coord):
                _cnt[0] += 1
                ti = wpool.tile(shp, I32, name=f"ct{_cnt[0]}")
                nc.any.tensor_copy(ti[:], i_coord[:])     # f32 -> i32 cast
                tf = t()
                nc.any.tensor_copy(tf[:], ti[:])          # i32 -> f32 cast
                g = t()
                nc.vector.tensor_tensor(g[:], tf[:], i_coord[:], is_gt)
                f0 = t()
                nc.vector.tensor_tensor(f0[:], tf[:], g[:], sub)
                return f0

            def axis_frac(off_t, base_t):
                # i = off*13.5 + base ; returns (i, floor(i))
                i_c = t()
                nc.vector.scalar_tensor_tensor(i_c[:], off_t[:], SC, base_t[:],
                                               mult, add)
                return i_c, floor_of(i_c)

            ix, x0f = axis_frac(offx, basex_sb)
            iy, y0f = axis_frac(offy, basey_sb)

            def frac_weights(i_c, f0):
                w1 = t()
                nc.vector.tensor_tensor(w1[:], i_c[:], f0[:], sub)
                w0 = t()
                nc.vector.tensor_scalar(w0[:], w1[:], -1.0, 1.0, mult, add)
                return w0, w1

            wx0, wx1 = frac_weights(ix, x0f)
            wy0, wy1 = frac_weights(iy, y0f)

            def clip_valid(f0):
                # returns (clip(f0), valid(f0), clip(f0+1), valid(f0+1))
                c0 = t()
                nc.vector.tensor_scalar(c0[:], f0[:], 27.0, 0.0, amin, amax)
                v0 = t()
                nc.vector.tensor_tensor(v0[:], c0[:], f0[:], is_eq)
                f1 = t()
                nc.vector.tensor_scalar_add(f1[:], f0[:], 1.0)
                c1 = t()
                nc.vector.tensor_scalar(c1[:], f1[:], 27.0, 0.0, amin, amax)
                v1 = t()
                nc.vector.tensor_tensor(v1[:], c1[:], f1[:], is_eq)
                return c0, v0, c1, v1

            x0c, vx0, x1c, vx1 = clip_valid(x0f)
            _, vy0, _, vy1 = clip_valid(y0f)

            # y pair-table row: s = clip(y0f, -1, 27) + 1 ; yb = s*28
            y0cp = t()
            nc.vector.tensor_scalar(y0cp[:], y0f[:], 27.0, -1.0, amin, amax)
            yb = t()
            nc.vector.tensor_scalar(yb[:], y0cp[:], float(W), float(W), mult, add)

            def vmul(a, b):
                o = t()
                nc.vector.tensor_tensor(o[:], a[:], b[:], mult)
                return o

            wx0v, wx1v = vmul(wx0, vx0), vmul(wx1, vx1)
            wy0v, wy1v = vmul(wy0, vy0), vmul(wy1, vy1)
            w00, w01 = vmul(wy0v, wx0v), vmul(wy0v, wx1v)
            w10, w11 = vmul(wy1v, wx0v), vmul(wy1v, wx1v)

            idxa_f = t()
            nc.vector.tensor_tensor(idxa_f[:], yb[:], x0c[:], add)
            idxb_f = t()
            nc.vector.tensor_tensor(idxb_f[:], yb[:], x1c[:], add)
            # ---- 3. wrap idx into dma_gather's 16-partition layout via
            # 8 permutation matmuls: wrap[q, m*8+g] = idx_f[g*16 + q%16, m]
            NI = 128 * NCH * K2  # 4608 gathered rows per tensor

            def wrap_idx(idx_f, tag):
                psw = opool.tile([128, 8, NCH * K2], F32, tag=f"psw{tag}",
                                 name=f"psw{tag}")
                for gsel in range(8):
                    nc.tensor.matmul(
                        out=psw[:, gsel, :], lhsT=mg_sb[:, gsel, :],
                        rhs=idx_f[:].rearrange("p a b -> p (a b)"),
                        start=True, stop=True)
                wrap = wpool.tile([128, NCH * K2, 8], mybir.dt.int16,
                                  name=f"wrap{tag}")
                nc.any.tensor_copy(wrap[:].rearrange("q m g -> q g m"), psw[:])
                return wrap

            wrapa = wrap_idx(idxa_f, "a")
            wrapb = wrap_idx(idxb_f, "b")

            # ---- gathers: row i = m*128 + pt -> ga[pt, m, :] ----
            ga = wpool.tile([128, NCH, K2, 2 * CIN], F32)
            nc.gpsimd.dma_gather(
                out_ap=ga[:].rearrange("p a k c -> p (a k) c"),
                in_ap=xpair.ap(),
                idxs_ap=wrapa[:].rearrange("q m g -> q (m g)"),
                num_idxs=NI, num_idxs_reg=NI, elem_size=2 * CIN,
                single_packet=False)
            gb = wpool.tile([128, NCH, K2, 2 * CIN], F32)
            nc.gpsimd.dma_gather(
                out_ap=gb[:].rearrange("p a k c -> p (a k) c"),
                in_ap=xpair.ap(),
                idxs_ap=wrapb[:].rearrange("q m g -> q (m g)"),
                num_idxs=NI, num_idxs_reg=NI, elem_size=2 * CIN,
                single_packet=False)

            # ---- 4. combine ----
            def bc(wt):
                return wt[:, :, :, None].to_broadcast([128, NCH, K2, CIN])

            samp_t = wpool.tile([128, NCH, CIN, K2], F32)  # m=(c,kk) inner
            samp = samp_t[:].rearrange("p a c k -> p a k c")
            tmp_t = wpool.tile([128, NCH, CIN, K2], F32)
            tmp = tmp_t[:].rearrange("p a c k -> p a k c")
            nc.vector.tensor_tensor(samp, ga[:, :, :, 0:CIN], bc(w00), mult)
            nc.vector.tensor_tensor(tmp, ga[:, :, :, CIN:], bc(w10), mult)
            nc.vector.tensor_tensor(samp, samp, tmp, add)
            nc.vector.tensor_tensor(tmp, gb[:, :, :, 0:CIN], bc(w01), mult)
            nc.vector.tensor_tensor(samp, samp, tmp, add)
            nc.vector.tensor_tensor(tmp, gb[:, :, :, CIN:], bc(w11), mult)
            nc.vector.tensor_tensor(samp, samp, tmp, add)

            # ---- 5. transpose to s-chunks [m(128), pt] ----
            NB = 5  # ceil(576/128)
            rhs = wpool.tile([128, NB, NPT], F32)
            # rows 64:128 of the last m-chunk are padding (576 -> 640): the
            # K=128 matmul reads them, so they must be zeroed (their weights
            # are zero, but NaN garbage would still poison the product).
            nc.any.memset(rhs[64:, NB - 1, :], 0.0)
            sv = samp_t[:].rearrange("p a c k -> p a (c k)")
            for ch in range(NCH):
                for b in range(NB):
                    mlo, mhi = 128 * b, min(128 * (b + 1), CIN * K2)
                    pst = ppool.tile([128, 128], F32, tag="tps")
                    nc.tensor.transpose(
                        pst[:mhi - mlo, :], sv[:, ch, mlo:mhi], ident[:])
                    nc.any.tensor_copy(
                        rhs[:mhi - mlo, b, ch * PCH:(ch + 1) * PCH],
                        pst[:mhi - mlo, :PCH])

            # ---- 6. final matmuls: ps1 = W~ @ s, ps2 = B~ @ s ----
            ps1 = popool.tile([COUT, NPT], F32, name="ps1")
            ps2 = popool.tile([COUT, NPT], F32, name="ps2")
            for b in range(NB):
                nc.tensor.matmul(
                    out=ps1[:], lhsT=wwb_sb[:, b, :], rhs=rhs[:, b, :],
                    start=(b == 0), stop=(b == NB - 1))
            for b in range(NB):
                nc.tensor.matmul(
                    out=ps2[:], lhsT=wwb_sb[:, NB + b, :], rhs=rhs[:, b, :],
                    start=(b == 0), stop=(b == NB - 1))
            out_sb = wpool.tile([COUT, NPT], F32)
            nc.vector.tensor_tensor(out_sb[:], ps1[:], xc_sb[:COUT, :], mult)
            nc.vector.tensor_tensor(out_sb[:], out_sb[:], ps2[:], add)
            nc.sync.dma_start(out_d.ap(), out_sb[:])

    nc.compile()
    return nc


def _host_inputs(x, w_off, b_off, w_wgt, b_wgt):
    """Build the 8 per-core input dicts (layout/shard prep only)."""
    x = np.asarray(x, dtype=np.float32)
    w_off = np.asarray(w_off, dtype=np.float32)
    b_off = np.asarray(b_off, dtype=np.float32)
    w_wgt = np.asarray(w_wgt, dtype=np.float32)
    b_wgt = np.asarray(b_wgt, dtype=np.float32)

    xs = np.linspace(-1.0, 1.0, W).astype(np.float32)
    ys = np.linspace(-1.0, 1.0, H).astype(np.float32)
    kx = np.linspace(-(K - 1) / (W - 1), (K - 1) / (W - 1), K).astype(np.float32)
    ky = np.linspace(-(K - 1) / (H - 1), (K - 1) / (H - 1), K).astype(np.float32)

    # wwb [128, 10, 64]: chunks 0..4 = W~.T (640x64, zero-padded from 576),
    # chunks 5..9 = B~.T, where W~ = w_wgt [64, 576], B~ = b_wgt.reshape(64, 576)
    wtp = np.zeros((640, COUT), dtype=np.float32)
    wtp[:576] = w_wgt.T
    btp = np.zeros((640, COUT), dtype=np.float32)
    btp[:576] = b_wgt.reshape(CIN, K2 * COUT).T
    wwb = np.concatenate([wtp.reshape(5, 128, COUT),
                          btp.reshape(5, 128, COUT)], axis=0)  # [10,128,64]
    wwb = wwb.transpose(1, 0, 2).reshape(128, 10 * COUT).copy()

    # idx-wrap permutation selectors: mg[pt, g*128+q] = (pt == g*16 + q%16)
    mg = np.zeros((128, 8, 128), dtype=np.float32)
    q = np.arange(128)
    for gsel in range(8):
        mg[gsel * 16 + (q % 16), gsel, q] = 1.0
    mg = mg.reshape(128, 8 * 128)

    wofft = np.zeros((128, 2 * K2), dtype=np.float32)
    wofft[:CIN] = w_off.T

    in_maps = []
    for c in range(NCORES):
        n, half = divmod(c, 2)
        r0 = HHALF * half
        xn = x[n]                             # [64, 28, 28]
        x_hwc = xn.transpose(1, 2, 0)         # [28, 28, 64]

        tbl = np.zeros((H + 1, W, 2 * CIN), dtype=np.float32)
        rt = np.clip(np.arange(H + 1) - 1, 0, H - 1)
        rb = np.clip(np.arange(H + 1), 0, H - 1)
        tbl[:, :, :CIN] = x_hwc[rt]
        tbl[:, :, CIN:] = x_hwc[rb]

        xcpad = np.zeros((128, NPT), dtype=np.float32)
        xcpad[:CIN] = xn.reshape(CIN, H * W)[:, r0 * W:r0 * W + NPT]

        # base grids [128, NCH, K2]
        bx = np.zeros((128, NCH, K2), dtype=np.float32)
        by = np.zeros((128, NCH, K2), dtype=np.float32)
        p_idx = np.arange(PCH)
        for ch in range(NCH):
            g = r0 * W + ch * PCH + p_idx          # global pixel
            row, col = g // W, g % W
            for kk in range(K2):
                kyi, kxi = divmod(kk, K)
                bx[:PCH, ch, kk] = (xs[col] + kx[kxi] + b_off[2 * kk] + 1.0) * SC
                by[:PCH, ch, kk] = (ys[row] + ky[kyi] + b_off[2 * kk + 1] + 1.0) * SC
        # pad rows: safe in-range coords (center pixel, zero offset)
        bx[PCH:] = SC
        by[PCH:] = SC

        in_maps.append({
            "xpair": tbl.reshape(TBL_ROWS, 2 * CIN),
            "xcpad": xcpad,
            "wofft": wofft,
            "basex": bx.reshape(128, NCH * K2),
            "basey": by.reshape(128, NCH * K2),
            "wwb": wwb,
            "mg": mg,
        })
    return in_maps


def get_program():
    if "nc" not in _CACHE:
        _CACHE["nc"] = _build_program()
    return _CACHE["nc"]


def run_cores(in_maps, **kw):
    nc = get_program()
    return run_bass_kernel_spmd(nc, in_maps, core_ids=list(range(NCORES)), **kw)


def assemble(results):
    out = np.zeros((N, COUT, H, W), dtype=np.float32)
    for c in range(NCORES):
        n, half = divmod(c, 2)
        out[n, :, HHALF * half:HHALF * (half + 1), :] = \
            results[c]["out"].reshape(COUT, HHALF, W)
    return out


def kernel(x, w_off, b_off, w_wgt, b_wgt):
    in_maps = _host_inputs(x, w_off, b_off, w_wgt, b_wgt)
    res = run_cores(in_maps)
    return assemble(res.results)

